# revision 19
# baseline (speedup 1.0000x reference)
"""Causal self-attention (B=2, T=2048, C=1024, H=16) on 8 TRN2 NeuronCores.

v3: hardware-loop (tc.For_i) implementation.  The backend charges a large
fixed cost per STATIC instruction; dynamic (looped) execution is ~free.  The
hot loops (contraction/k-tile/chunk) are hardware loops with register-indexed
(ds) slices, so the per-rep marginal cost is true silicon throughput.  The
reps loop is itself a hardware loop.

Constraints honoured: matmul stationaries must be register-free (staged via
copies), and any register-offset AP must sit at partition base 0 (the AP
resolver mis-handles nonzero bases), so b/qc/h are unrolled statically and
per-head k/q rows are restaged to base-0 tiles with partition-shifting DMAs.

Sharding: tensor-parallel over heads — each core owns 2 heads: qkv (f32) ->
causal attention (f32 scores, exp->bf16, multiplicative bf16 sliding causal
mask, bf16 AV with ones-column denominators) -> 1/denom ones-matmul
broadcast -> partial projection (bf16); host sums the 8 partials.
"""

import numpy as np
import ml_dtypes
from contextlib import ExitStack

import concourse.bass as bass
import concourse.tile as tile
from concourse import bacc, mybir
from concourse.bass import ds
from concourse.bass_utils import run_bass_kernel_spmd

BF16 = mybir.dt.bfloat16
F32 = mybir.dt.float32
Exp = mybir.ActivationFunctionType.Exp
Mult = mybir.AluOpType.mult

B, T, C, H, D = 2, 2048, 1024, 16, 64
N_CORES = 8
HPC = H // N_CORES          # heads per core (2)
BT = B * T                  # 4096
NCT = C // 128              # contraction tiles (8)
NCH = BT // 512             # 512-token chunks (8)
AVS = 192                   # av block stride per k-tile
SCALE = 1.0 / np.sqrt(D)


def build_program(reps: int = 1):
    nc = bacc.Bacc("TRN2", target_bir_lowering=False, debug=False,
                   enable_asserts=True, num_devices=N_CORES)

    xT_d = nc.dram_tensor("xT", [C, BT], F32, kind="ExternalInput").ap()
    w_d = nc.dram_tensor("wqkv", [128, 3 * NCT * 128], F32,
                         kind="ExternalInput").ap()
    b_d = nc.dram_tensor("bqkv", [128, 3], F32, kind="ExternalInput").ap()
    wo_d = nc.dram_tensor("wo", [128, NCT * 128], BF16,
                          kind="ExternalInput").ap()
    m01_d = nc.dram_tensor("m01", [128, 4 * 512], BF16,
                           kind="ExternalInput").ap()
    ones2_d = nc.dram_tensor("ones2", [2, 128], F32, kind="ExternalInput").ap()
    out_d = nc.dram_tensor("outT", [C, BT], BF16, kind="ExternalOutput").ap()

    xT3 = xT_d.rearrange("(a p) t -> p a t", p=128)      # [128, 8, 4096]
    out3 = out_d.rearrange("(a p) t -> p a t", p=128)    # [128, 8, 4096]

    with tile.TileContext(nc) as tc, ExitStack() as ctx:
        sb = ctx.enter_context(tc.tile_pool(name="sb", bufs=1))
        ps = ctx.enter_context(tc.tile_pool(name="ps", bufs=1, space="PSUM"))

        xck = sb.tile([128, NCT * 512], F32, tag="xck")       # 16KB
        q_sb = sb.tile([128, BT], F32, tag="q_sb")            # 16KB
        k_sb = sb.tile([128, BT], F32, tag="k_sb")            # 16KB
        v_b = sb.tile([128, BT], BF16, tag="v_b")             # 8KB
        qh = sb.tile([64, BT], F32, tag="qh")                 # 16KB
        kh = sb.tile([64, BT], F32, tag="kh")                 # 16KB
        av_b = sb.tile([128, 32 * AVS], BF16, tag="av_b")     # 12KB
        p4 = sb.tile([128, 2048], BF16, tag="p4")             # 4KB
        y2 = sb.tile([128, BT], F32, tag="y2")                # 16KB
        y2b = sb.tile([128, BT], BF16, tag="y2b")             # 8KB
        dn = sb.tile([128, BT], F32, tag="dn")                # 16KB
        rcp = sb.tile([2, BT], F32, tag="rcp")                # 16KB
        m01 = sb.tile([128, 4 * 512], BF16, tag="m01")        # 4KB
        wsb = sb.tile([128, 3 * NCT * 128], F32, tag="wsb")   # 12KB
        wo = sb.tile([128, NCT * 128], BF16, tag="wo")        # 4KB
        bqkv = sb.tile([128, 3], F32, tag="bqkv")
        ones2 = sb.tile([2, 128], F32, tag="ones2")
        ob = sb.tile([128, 1024], BF16, tag="ob")
        kstat = sb.tile([64, 512], F32, tag="kstat")
        avstat = sb.tile([128, 4 * AVS], BF16, tag="avstat")

        qkvps = ps.tile([128, 512], F32, tag="qkvps")
        s_ps = ps.tile([128, 2048], F32, tag="s_ps")
        y_ps0 = ps.tile([128, 512], F32, tag="y_ps0")
        y_ps1 = ps.tile([128, 512], F32, tag="y_ps1")
        o_ps = ps.tile([128, 512], F32, tag="o_ps")

        # ---- setup (once per program) ----
        nc.sync.dma_start(wsb[:], w_d)
        nc.sync.dma_start(wo[:], wo_d)
        nc.sync.dma_start(bqkv[:], b_d)
        nc.sync.dma_start(m01[:], m01_d)
        nc.sync.dma_start(ones2[:], ones2_d)
        # av_b block: [dA(0:64) | 1(64) | 0(65:128) | dB(128:192)]
        nc.vector.memset(av_b[:], 0.0)
        nc.vector.memset(
            av_b[:].rearrange("p (b f) -> p b f", f=AVS)[:, :, 64:65], 1.0)

        with tc.For_i(0, reps) as _rep:
            # ---- P1: qkv = W.T @ x, x streamed per 512-token chunk ----
            with tc.For_i(0, NCH, 2) as ch:
                for half in range(2):
                    nc.sync.dma_start(
                        xck[:].rearrange("p (a t) -> p a t", t=512),
                        xT3[:, :, ds(ch * 512 + half * 512, 512)])
                    for g, dst in ((0, q_sb), (1, k_sb), (2, v_b)):
                        for ct in range(NCT):
                            nc.tensor.matmul(
                                qkvps[:],
                                wsb[:, (g * NCT + ct) * 128:
                                    (g * NCT + ct + 1) * 128],
                                xck[:, ct * 512:(ct + 1) * 512],
                                start=(ct == 0), stop=(ct == NCT - 1))
                        nc.vector.tensor_scalar_add(
                            dst[:, ds(ch * 512 + half * 512, 512)], qkvps[:],
                            bqkv[:, g:g + 1])

            # ---- P1.5: block-transpose v into av_b ----
            av3 = av_b[:].rearrange("p (b f) -> p b f", f=AVS)
            nc.sync.dma_start_transpose(av3[:, :, 0:64], v_b[0:64, :])
            nc.sync.dma_start_transpose(av3[:, :, 128:192], v_b[64:128, :])

            # ---- P2: causal attention ----
            for h in range(HPC):
                hsl = slice(h * 64, (h + 1) * 64)
                # restage this head's q/k rows at partition base 0
                nc.sync.dma_start(qh[:], q_sb[hsl, :])
                nc.sync.dma_start(kh[:], k_sb[hsl, :])
                for qc in range(4):
                    nc.vector.memset(y_ps0[:], 0.0)
                    nc.vector.memset(y_ps1[:], 0.0)
                    # sub-diagonal groups, both batches per iteration
                    with tc.For_i(0, qc * 4, 4) as kt:
                        for b, y_ps in ((0, y_ps0), (1, y_ps1)):
                            bcol = b * 2048 + qc * 512
                            nc.vector.tensor_copy(
                                kstat[:], kh[:, ds(b * 2048 + kt * 128, 512)])
                            for u in range(4):
                                nc.tensor.matmul(
                                    s_ps[:, u * 512:(u + 1) * 512],
                                    kstat[:, u * 128:(u + 1) * 128],
                                    qh[:, bcol:bcol + 512],
                                    start=True, stop=True)
                            nc.scalar.activation(
                                p4[:], s_ps[:], Exp, scale=SCALE)
                            nc.vector.tensor_copy(
                                avstat[:],
                                av_b[:, ds(kt * AVS + b * 16 * AVS, 4 * AVS)])
                            for u in range(4):
                                nc.tensor.matmul(
                                    y_ps[:],
                                    avstat[:, u * AVS + h * 64:
                                           u * AVS + h * 64 + 128],
                                    p4[:, u * 512:(u + 1) * 512],
                                    start=False, stop=False,
                                    skip_group_check=True)
                    # diagonal groups (kt = 4qc..4qc+3): static, masked
                    for b, y_ps in ((0, y_ps0), (1, y_ps1)):
                        bcol = b * 2048 + qc * 512
                        nc.vector.tensor_copy(
                            kstat[:], kh[:, bcol:bcol + 512])
                        for u in range(4):
                            nc.tensor.matmul(
                                s_ps[:, u * 512:(u + 1) * 512],
                                kstat[:, u * 128:(u + 1) * 128],
                                qh[:, bcol:bcol + 512],
                                start=True, stop=True)
                        nc.scalar.activation(
                            p4[:], s_ps[:], Exp, scale=SCALE)
                        nc.vector.tensor_tensor(p4[:], p4[:], m01[:], op=Mult)
                        da = (b * 16 + qc * 4) * AVS
                        nc.vector.tensor_copy(
                            avstat[:], av_b[:, da:da + 4 * AVS])
                        for u in range(4):
                            nc.tensor.matmul(
                                y_ps[:],
                                avstat[:, u * AVS + h * 64:
                                       u * AVS + h * 64 + 128],
                                p4[:, u * 512:(u + 1) * 512],
                                start=False, stop=False,
                                skip_group_check=True)
                        if h == 0:
                            nc.vector.tensor_copy(
                                y2[0:64, bcol:bcol + 512], y_ps[0:64, :])
                            nc.vector.tensor_copy(
                                dn[64:65, bcol:bcol + 512], y_ps[64:65, :])
                        else:
                            nc.vector.tensor_copy(
                                y2[64:128, bcol:bcol + 512], y_ps[64:128, :])
                            nc.vector.tensor_copy(
                                dn[0:1, bcol:bcol + 512], y_ps[0:1, :])

            # ---- P3: y2 /= denom (ones-matmul partition broadcast) ----
            nc.sync.dma_start(rcp[0:1, :], dn[64:65, :])
            nc.sync.dma_start(rcp[1:2, :], dn[0:1, :])
            nc.vector.reciprocal_approx_fast(rcp[:], rcp[:])
            for ch in range(NCH):
                cc = ch * 512
                nc.tensor.matmul(o_ps[0:64, :], ones2[:, 0:64],
                                 rcp[:, cc:cc + 512], start=True, stop=True)
                nc.tensor.matmul(o_ps[64:128, :], ones2[:, 64:128],
                                 rcp[:, cc:cc + 512], start=True, stop=True)
                nc.vector.tensor_tensor(
                    y2b[:, cc:cc + 512], y2[:, cc:cc + 512], o_ps[:],
                    op=Mult)

            # ---- P4: out_T partial = Wproj_h.T @ y2b ----
            with tc.For_i(0, NCH, 2) as ch:
                for ct in range(NCT):
                    for u in range(2):
                        nc.tensor.matmul(
                            o_ps[:], wo[:, ct * 128:(ct + 1) * 128],
                            y2b[:, ds(ch * 512 + u * 512, 512)],
                            start=True, stop=True)
                        nc.vector.tensor_copy(
                            ob[:, u * 512:(u + 1) * 512], o_ps[:])
                    nc.sync.dma_start(
                        out3[:, ct:ct + 1, ds(ch * 512, 1024)], ob[:])

    nc.compile()
    return nc


def make_in_maps(x, Wqkv, bqkv, Wproj):
    """Host-side sharding: per-core input dict."""
    bf = ml_dtypes.bfloat16
    xT = np.ascontiguousarray(x.reshape(BT, C).T).astype(np.float32)
    # diagonal-group multiplicative 0/1 causal mask, j = 0..3 sub-tiles:
    # zero iff cq < j*128 + r
    r = np.arange(128)[:, None]
    cq = np.arange(512)[None, :]
    m01 = np.zeros((128, 4 * 512), np.float32)
    for j in range(4):
        m01[:, j * 512:(j + 1) * 512] = np.where(cq < j * 128 + r, 0.0, 1.0)
    ones2 = np.zeros((2, 128), np.float32)
    ones2[0, 0:64] = 1.0
    ones2[1, 64:128] = 1.0
    in_maps = []
    for c in range(N_CORES):
        h0 = c * HPC
        cols = np.r_[h0 * D:(h0 + 2) * D]          # this core's 128 features
        wq = Wqkv[:, cols]
        wk = Wqkv[:, C + cols]
        wv = Wqkv[:, 2 * C + cols]
        wsb = np.zeros((128, 3 * NCT * 128), np.float32)
        for g, wg in enumerate((wq, wk, wv)):
            for ct in range(NCT):
                wsb[:, (g * NCT + ct) * 128:(g * NCT + ct + 1) * 128] = \
                    wg[ct * 128:(ct + 1) * 128, :]
        bq = np.zeros((128, 3), np.float32)
        bq[:, 0] = bqkv[cols]
        bq[:, 1] = bqkv[C + cols]
        bq[:, 2] = bqkv[2 * C + cols]
        wob = np.zeros((128, NCT * 128), np.float32)
        for ct in range(NCT):
            wob[:, ct * 128:(ct + 1) * 128] = \
                Wproj[cols, ct * 128:(ct + 1) * 128]
        in_maps.append({
            "xT": xT,
            "wqkv": wsb,
            "bqkv": bq,
            "wo": wob.astype(bf),
            "m01": m01.astype(bf),
            "ones2": ones2,
        })
    return in_maps


_PROG = None


def _get_prog():
    global _PROG
    if _PROG is None:
        _PROG = build_program(reps=1)
    return _PROG


def kernel(x, Wqkv, bqkv, Wproj, bproj):
    x = np.asarray(x, dtype=np.float32)
    Wqkv = np.asarray(Wqkv, dtype=np.float32)
    bqkv = np.asarray(bqkv, dtype=np.float32)
    Wproj = np.asarray(Wproj, dtype=np.float32)
    bproj = np.asarray(bproj, dtype=np.float32)

    nc = _get_prog()
    in_maps = make_in_maps(x, Wqkv, bqkv, Wproj)
    res = run_bass_kernel_spmd(nc, in_maps, core_ids=list(range(N_CORES)))
    acc = np.zeros((C, BT), dtype=np.float32)
    for c in range(N_CORES):
        acc += res.results[c]["outT"].astype(np.float32)
    out = acc.T + bproj[None, :]
    return np.ascontiguousarray(out.reshape(B, T, C), dtype=np.float32)


# revision 20
# speedup vs baseline: 1.3467x; 1.3467x over previous
"""Causal self-attention (B=2, T=2048, C=1024, H=16) on 8 TRN2 NeuronCores.

v3: hardware-loop (tc.For_i) implementation.  The backend charges a large
fixed cost per STATIC instruction; dynamic (looped) execution is ~free.  The
hot loops (contraction/k-tile/chunk) are hardware loops with register-indexed
(ds) slices, so the per-rep marginal cost is true silicon throughput.  The
reps loop is itself a hardware loop.

Constraints honoured: matmul stationaries must be register-free (staged via
copies), and any register-offset AP must sit at partition base 0 (the AP
resolver mis-handles nonzero bases), so b/qc/h are unrolled statically and
per-head k/q rows are restaged to base-0 tiles with partition-shifting DMAs.

Sharding: tensor-parallel over heads — each core owns 2 heads: qkv (f32) ->
causal attention (f32 scores, exp->bf16, multiplicative bf16 sliding causal
mask, bf16 AV with ones-column denominators) -> 1/denom ones-matmul
broadcast -> partial projection (bf16); host sums the 8 partials.
"""

import numpy as np
import ml_dtypes
from contextlib import ExitStack

import concourse.bass as bass
import concourse.tile as tile
from concourse import bacc, mybir
from concourse.bass import ds
from concourse.bass_utils import run_bass_kernel_spmd

BF16 = mybir.dt.bfloat16
F32 = mybir.dt.float32
Exp = mybir.ActivationFunctionType.Exp
Mult = mybir.AluOpType.mult

B, T, C, H, D = 2, 2048, 1024, 16, 64
N_CORES = 8
HPC = H // N_CORES          # heads per core (2)
BT = B * T                  # 4096
NCT = C // 128              # contraction tiles (8)
NCH = BT // 512             # 512-token chunks (8)
AVS = 192                   # av block stride per k-tile
SCALE = 1.0 / np.sqrt(D)


def build_program(reps: int = 1):
    nc = bacc.Bacc("TRN2", target_bir_lowering=False, debug=False,
                   enable_asserts=True, num_devices=N_CORES)

    xT_d = nc.dram_tensor("xT", [C, BT], F32, kind="ExternalInput").ap()
    w_d = nc.dram_tensor("wqkv", [128, 3 * NCT * 128], F32,
                         kind="ExternalInput").ap()
    b_d = nc.dram_tensor("bqkv", [128, 3], F32, kind="ExternalInput").ap()
    wo_d = nc.dram_tensor("wo", [128, NCT * 128], BF16,
                          kind="ExternalInput").ap()
    m01_d = nc.dram_tensor("m01", [128, 4 * 512], BF16,
                           kind="ExternalInput").ap()
    ones2_d = nc.dram_tensor("ones2", [2, 128], F32, kind="ExternalInput").ap()
    out_d = nc.dram_tensor("outT", [C, BT], BF16, kind="ExternalOutput").ap()

    xT3 = xT_d.rearrange("(a p) t -> p a t", p=128)      # [128, 8, 4096]
    out3 = out_d.rearrange("(a p) t -> p a t", p=128)    # [128, 8, 4096]

    with tile.TileContext(nc) as tc, ExitStack() as ctx:
        sb = ctx.enter_context(tc.tile_pool(name="sb", bufs=1))
        ps = ctx.enter_context(tc.tile_pool(name="ps", bufs=1, space="PSUM"))

        xck = sb.tile([128, NCT * 512], F32, tag="xck")       # 16KB
        q_sb = sb.tile([128, BT], F32, tag="q_sb")            # 16KB
        k_sb = sb.tile([128, BT], F32, tag="k_sb")            # 16KB
        v_b = sb.tile([128, BT], BF16, tag="v_b")             # 8KB
        qh = sb.tile([64, BT], F32, tag="qh")                 # 16KB
        kh = sb.tile([64, BT], F32, tag="kh")                 # 16KB
        av_b = sb.tile([128, 32 * AVS], BF16, tag="av_b")     # 12KB
        p4 = sb.tile([128, 2048], BF16, tag="p4")             # 4KB
        y2 = sb.tile([128, BT], F32, tag="y2")                # 16KB
        y2b = sb.tile([128, BT], BF16, tag="y2b")             # 8KB
        dn = sb.tile([128, BT], F32, tag="dn")                # 16KB
        rcp = sb.tile([2, BT], F32, tag="rcp")                # 16KB
        m01 = sb.tile([128, 4 * 512], BF16, tag="m01")        # 4KB
        wsb = sb.tile([128, 3 * NCT * 128], F32, tag="wsb")   # 12KB
        wo = sb.tile([128, NCT * 128], BF16, tag="wo")        # 4KB
        bqkv = sb.tile([128, 3], F32, tag="bqkv")
        ones2 = sb.tile([2, 128], F32, tag="ones2")
        ob = sb.tile([128, 1024], BF16, tag="ob")
        kstat = sb.tile([64, 512], F32, tag="kstat")
        avstat = sb.tile([128, 4 * AVS], BF16, tag="avstat")

        qkvps = ps.tile([128, 512], F32, tag="qkvps")
        s_ps = ps.tile([128, 2048], F32, tag="s_ps")
        y_ps0 = ps.tile([128, 512], F32, tag="y_ps0")
        y_ps1 = ps.tile([128, 512], F32, tag="y_ps1")
        o_ps = ps.tile([128, 512], F32, tag="o_ps")

        # ---- setup (once per program) ----
        nc.sync.dma_start(wsb[:], w_d)
        nc.sync.dma_start(wo[:], wo_d)
        nc.sync.dma_start(bqkv[:], b_d)
        nc.sync.dma_start(m01[:], m01_d)
        nc.sync.dma_start(ones2[:], ones2_d)
        # av_b block: [dA(0:64) | 1(64) | 0(65:128) | dB(128:192)]
        nc.vector.memset(av_b[:], 0.0)
        nc.vector.memset(
            av_b[:].rearrange("p (b f) -> p b f", f=AVS)[:, :, 64:65], 1.0)

        with tc.For_i(0, reps) as _rep:
            # ---- P1: qkv = W.T @ x, x streamed per 512-token chunk ----
            with tc.For_i(0, NCH, 2) as ch:
                for half in range(2):
                    nc.sync.dma_start(
                        xck[:].rearrange("p (a t) -> p a t", t=512),
                        xT3[:, :, ds(ch * 512 + half * 512, 512)])
                    for g, dst in ((0, q_sb), (1, k_sb), (2, v_b)):
                        for ct in range(NCT):
                            nc.tensor.matmul(
                                qkvps[:],
                                wsb[:, (g * NCT + ct) * 128:
                                    (g * NCT + ct + 1) * 128],
                                xck[:, ct * 512:(ct + 1) * 512],
                                start=(ct == 0), stop=(ct == NCT - 1))
                        nc.vector.tensor_scalar_add(
                            dst[:, ds(ch * 512 + half * 512, 512)], qkvps[:],
                            bqkv[:, g:g + 1])

            # ---- P1.5: block-transpose v into av_b ----
            av3 = av_b[:].rearrange("p (b f) -> p b f", f=AVS)
            nc.sync.dma_start_transpose(av3[:, :, 0:64], v_b[0:64, :])
            nc.sync.dma_start_transpose(av3[:, :, 128:192], v_b[64:128, :])

            # ---- P2: causal attention ----
            for h in range(HPC):
                hsl = slice(h * 64, (h + 1) * 64)
                # restage this head's q/k rows at partition base 0
                nc.sync.dma_start(qh[:], q_sb[hsl, :])
                nc.sync.dma_start(kh[:], k_sb[hsl, :])
                for qc in range(4):
                    nc.vector.memset(y_ps0[:], 0.0)
                    nc.vector.memset(y_ps1[:], 0.0)
                    # sub-diagonal groups, both batches per iteration
                    with tc.For_i(0, qc * 4, 4) as kt:
                        for b, y_ps in ((0, y_ps0), (1, y_ps1)):
                            bcol = b * 2048 + qc * 512
                            nc.vector.tensor_copy(
                                kstat[:], kh[:, ds(b * 2048 + kt * 128, 512)])
                            for u in range(4):
                                nc.tensor.matmul(
                                    s_ps[:, u * 512:(u + 1) * 512],
                                    kstat[:, u * 128:(u + 1) * 128],
                                    qh[:, bcol:bcol + 512],
                                    start=True, stop=True)
                            nc.scalar.activation(
                                p4[:], s_ps[:], Exp, scale=SCALE)
                            nc.vector.tensor_copy(
                                avstat[:],
                                av_b[:, ds(kt * AVS + b * 16 * AVS, 4 * AVS)])
                            for u in range(4):
                                nc.tensor.matmul(
                                    y_ps[:],
                                    avstat[:, u * AVS + h * 64:
                                           u * AVS + h * 64 + 128],
                                    p4[:, u * 512:(u + 1) * 512],
                                    start=False, stop=False,
                                    skip_group_check=True)
                    # diagonal groups (kt = 4qc..4qc+3): static, masked.
                    # All offsets static -> matmuls read kh/av_b directly,
                    # no staging copies.
                    for b, y_ps in ((0, y_ps0), (1, y_ps1)):
                        bcol = b * 2048 + qc * 512
                        for u in range(4):
                            nc.tensor.matmul(
                                s_ps[:, u * 512:(u + 1) * 512],
                                kh[:, bcol + u * 128:bcol + (u + 1) * 128],
                                qh[:, bcol:bcol + 512],
                                start=True, stop=True)
                        nc.scalar.activation(
                            p4[:], s_ps[:], Exp, scale=SCALE)
                        nc.vector.tensor_tensor(p4[:], p4[:], m01[:], op=Mult)
                        da = (b * 16 + qc * 4) * AVS + h * 64
                        for u in range(4):
                            nc.tensor.matmul(
                                y_ps[:],
                                av_b[:, da + u * AVS:da + u * AVS + 128],
                                p4[:, u * 512:(u + 1) * 512],
                                start=False, stop=False,
                                skip_group_check=True)
                        if h == 0:
                            nc.vector.tensor_copy(
                                y2[0:64, bcol:bcol + 512], y_ps[0:64, :])
                            nc.vector.tensor_copy(
                                dn[64:65, bcol:bcol + 512], y_ps[64:65, :])
                        else:
                            nc.vector.tensor_copy(
                                y2[64:128, bcol:bcol + 512], y_ps[64:128, :])
                            nc.vector.tensor_copy(
                                dn[0:1, bcol:bcol + 512], y_ps[0:1, :])

            # ---- P3: y2 /= denom (ones-matmul partition broadcast) ----
            nc.sync.dma_start(rcp[0:1, :], dn[64:65, :])
            nc.sync.dma_start(rcp[1:2, :], dn[0:1, :])
            nc.vector.reciprocal_approx_fast(rcp[:], rcp[:])
            for ch in range(NCH):
                cc = ch * 512
                nc.tensor.matmul(o_ps[0:64, :], ones2[:, 0:64],
                                 rcp[:, cc:cc + 512], start=True, stop=True)
                nc.tensor.matmul(o_ps[64:128, :], ones2[:, 64:128],
                                 rcp[:, cc:cc + 512], start=True, stop=True)
                nc.vector.tensor_tensor(
                    y2b[:, cc:cc + 512], y2[:, cc:cc + 512], o_ps[:],
                    op=Mult)

            # ---- P4: out_T partial = Wproj_h.T @ y2b ----
            with tc.For_i(0, NCH, 2) as ch:
                for ct in range(NCT):
                    for u in range(2):
                        nc.tensor.matmul(
                            o_ps[:], wo[:, ct * 128:(ct + 1) * 128],
                            y2b[:, ds(ch * 512 + u * 512, 512)],
                            start=True, stop=True)
                        nc.vector.tensor_copy(
                            ob[:, u * 512:(u + 1) * 512], o_ps[:])
                    nc.sync.dma_start(
                        out3[:, ct:ct + 1, ds(ch * 512, 1024)], ob[:])

    nc.compile()
    return nc


def make_in_maps(x, Wqkv, bqkv, Wproj):
    """Host-side sharding: per-core input dict."""
    bf = ml_dtypes.bfloat16
    xT = np.ascontiguousarray(x.reshape(BT, C).T).astype(np.float32)
    # diagonal-group multiplicative 0/1 causal mask, j = 0..3 sub-tiles:
    # zero iff cq < j*128 + r
    r = np.arange(128)[:, None]
    cq = np.arange(512)[None, :]
    m01 = np.zeros((128, 4 * 512), np.float32)
    for j in range(4):
        m01[:, j * 512:(j + 1) * 512] = np.where(cq < j * 128 + r, 0.0, 1.0)
    ones2 = np.zeros((2, 128), np.float32)
    ones2[0, 0:64] = 1.0
    ones2[1, 64:128] = 1.0
    in_maps = []
    for c in range(N_CORES):
        h0 = c * HPC
        cols = np.r_[h0 * D:(h0 + 2) * D]          # this core's 128 features
        wq = Wqkv[:, cols]
        wk = Wqkv[:, C + cols]
        wv = Wqkv[:, 2 * C + cols]
        wsb = np.zeros((128, 3 * NCT * 128), np.float32)
        for g, wg in enumerate((wq, wk, wv)):
            for ct in range(NCT):
                wsb[:, (g * NCT + ct) * 128:(g * NCT + ct + 1) * 128] = \
                    wg[ct * 128:(ct + 1) * 128, :]
        bq = np.zeros((128, 3), np.float32)
        bq[:, 0] = bqkv[cols]
        bq[:, 1] = bqkv[C + cols]
        bq[:, 2] = bqkv[2 * C + cols]
        wob = np.zeros((128, NCT * 128), np.float32)
        for ct in range(NCT):
            wob[:, ct * 128:(ct + 1) * 128] = \
                Wproj[cols, ct * 128:(ct + 1) * 128]
        in_maps.append({
            "xT": xT,
            "wqkv": wsb,
            "bqkv": bq,
            "wo": wob.astype(bf),
            "m01": m01.astype(bf),
            "ones2": ones2,
        })
    return in_maps


_PROG = None


def _get_prog():
    global _PROG
    if _PROG is None:
        _PROG = build_program(reps=1)
    return _PROG


def kernel(x, Wqkv, bqkv, Wproj, bproj):
    x = np.asarray(x, dtype=np.float32)
    Wqkv = np.asarray(Wqkv, dtype=np.float32)
    bqkv = np.asarray(bqkv, dtype=np.float32)
    Wproj = np.asarray(Wproj, dtype=np.float32)
    bproj = np.asarray(bproj, dtype=np.float32)

    nc = _get_prog()
    in_maps = make_in_maps(x, Wqkv, bqkv, Wproj)
    res = run_bass_kernel_spmd(nc, in_maps, core_ids=list(range(N_CORES)))
    acc = np.zeros((C, BT), dtype=np.float32)
    for c in range(N_CORES):
        acc += res.results[c]["outT"].astype(np.float32)
    out = acc.T + bproj[None, :]
    return np.ascontiguousarray(out.reshape(B, T, C), dtype=np.float32)
